# revision 4
# baseline (speedup 1.0000x reference)
"""Trainium2 Bass kernel for a 2-layer dense GCN block:

    z = x.reshape(B, N, F)                     # B=4, N=8192, F=64
    for i in range(2):
        z = relu((A @ z) @ W_i)                # A: [N, N] dense
    return z

Strategy (8 NeuronCores, SPMD):
  * Shard the output rows (m) of A @ Z across cores: core j owns rows
    [1024*j, 1024*(j+1)).  The host hands core j the matching
    column-slice of A^T (contraction dim n on SBUF partitions), cast to
    bf16 and pre-swizzled m-quarter-major so every DMA chunk is fully
    contiguous in DRAM.  The 16 MB shard stays resident in SBUF for
    BOTH layers -- A is read from HBM exactly once, at full ring rate
    on the sync HWDGE ring (z0 / z1 traffic uses the scalar ring).
  * Z is a [n, c] matrix with c = b*F + f (256 columns).  Layer matmuls
    compute H^T[c, m] = sum_n Z[n, c] * A^T[n, m] on the tensor engine
    (lhsT = Z tile stationary, rhs = A^T tile moving, fp32 PSUM accum).
    The n-loop is outermost, in DMA-chunk order.
  * Layer 1 runs as FOUR m-quarter passes (256 m columns each).  A
    quarter's rows finish mid-layer, so its AllGather (the only
    inter-layer exchange) triggers as early as ~1/4 through layer 1 and
    the whole 4-slice gather chain pipelines under the rest of layer 1
    and the start of layer 2 -- the collective's ~20us ncfw latency is
    completely hidden and the PE never idles between layers (idle >
    ~3.4us would also drop the HAM clock gate to half rate).
  * Gather slices land in their own SBUF tiles (z0 stays live), and the
    reload DMAs ride the sync ring so a reload waiting on its collective
    can never block a later z1_loc store (scalar ring) whose gather
    trigger is on the critical path.
  * Layer 2 accumulation matmuls are emitted at LOW scheduler priority
    so layer-1 tail work (weight-apply -> relu -> store -> gather
    trigger) always precedes them in the PE queue -- otherwise a
    reload-waiting LDWEIGHTS can wedge the whole PE FIFO.
  * Layer 2 consumes gather slices in arrival order, in two m-half
    passes whose output stores overlap the remaining matmuls.
  * bf16 operands / fp32 accumulation throughout (~0.5% rel-l2 vs the
    fp32 reference).
"""

import contextlib

import numpy as np
import ml_dtypes

import concourse.mybir as mybir
import concourse.tile as tile
from concourse import bacc
from concourse.bass_utils import run_bass_kernel_spmd

BF16 = ml_dtypes.bfloat16

NCORES = 8
B, N, F, L = 4, 8192, 64, 2
C = B * F                      # 256 columns of the Z matrix
M_CORE = N // NCORES           # 1024 output rows per core
NT = N // 128                  # 64 contraction tiles of 128
MT = M_CORE // 128             # 8 output-row tiles of 128 per core
KCH = 8                        # DMA chunks for the resident A^T shard
TPC = NT // KCH                # 8 n-tiles per chunk
NQ = 4                         # m-quarter passes / gather slices
MPG = MT // NQ                 # m-tiles per gather slice (2)
MQ = M_CORE // NQ              # m columns per quarter (256)

_CACHED = {}


def _build_program():
    nc = bacc.Bacc("TRN2", target_bir_lowering=False, debug=False,
                   num_devices=NCORES)
    dt = mybir.dt

    at_d = nc.dram_tensor("at", [NQ * N, MQ], dt.bfloat16, kind="ExternalInput")
    z0_d = nc.dram_tensor("z0", [N, C], dt.bfloat16, kind="ExternalInput")
    w_d = nc.dram_tensor("w", [128, 2 * 128], dt.bfloat16, kind="ExternalInput")
    out_d = nc.dram_tensor("out", [M_CORE, C], dt.bfloat16, kind="ExternalOutput")

    z1_loc = nc.dram_tensor("z1_loc", [M_CORE, C], dt.bfloat16)
    warm_in = nc.dram_tensor("warm_in", [MPG * 128, C], dt.bfloat16)
    warm_out = nc.dram_tensor("warm_out", [NCORES * MPG * 128, C], dt.bfloat16)
    z1g = [nc.dram_tensor(f"z1g{g}", [NCORES * MPG * 128, C], dt.bfloat16)
           for g in range(NQ)]

    # DRAM views with the n-tile index split out: [.., chunk, p, tile, cols]
    at_v = at_d.ap().rearrange("(q k t p) m -> q k p t m",
                               q=NQ, k=KCH, p=128)
    z0_v = z0_d.ap().rearrange("(k t p) c -> k p t c", k=KCH, p=128)

    with tile.TileContext(nc) as tc:
        with tc.tile_pool(name="a_res", bufs=1) as a_pool, \
             tc.tile_pool(name="z_res", bufs=1) as z_pool, \
             tc.tile_pool(name="z1_res", bufs=1) as z1_pool, \
             tc.tile_pool(name="wk", bufs=1) as w_pool, \
             tc.tile_pool(name="ht", bufs=1, space="PSUM") as psh_pool, \
             tc.tile_pool(name="pz", bufs=4, space="PSUM") as psz_pool, \
             tc.tile_pool(name="hsb", bufs=2) as hsb_pool, \
             tc.tile_pool(name="zout", bufs=8) as zout_pool:

            w_sb = w_pool.tile([128, 2 * 128], dt.bfloat16, tag="w")
            nc.scalar.dma_start(out=w_sb[:], in_=w_d[:])

            # Resident A^T shard: one SBUF tile per (quarter, chunk).
            at_sb = [[a_pool.tile([128, TPC * MQ], dt.bfloat16,
                                  tag=f"at{q}_{k}", name=f"at_sb{q}_{k}")
                      for k in range(KCH)] for q in range(NQ)]
            z_sb = [z_pool.tile([128, TPC * C], dt.bfloat16,
                                tag=f"z{k}", name=f"z_sb{k}")
                    for k in range(KCH)]
            z1_sb = [z1_pool.tile([128, NCORES * MPG * C], dt.bfloat16,
                                  tag=f"z1g{g}", name=f"z1_sb{g}")
                     for g in range(NQ)]

            def free3(tile_ap, inner):
                return tile_ap.rearrange("p (t i) -> p t i", i=inner)

            for k in range(KCH):
                nc.scalar.dma_start(out=free3(z_sb[k][:], C), in_=z0_v[k])
            for q in range(NQ):
                for k in range(KCH):
                    nc.sync.dma_start(out=free3(at_sb[q][k][:], MQ),
                                      in_=at_v[q][k])

            def z_tile(t, ch):
                """lhsT: Z[n-tile t, c-half ch] -> [128, 128] bf16."""
                k, tt = divmod(t, TPC)
                return z_sb[k][:, tt * C + ch * 128: tt * C + ch * 128 + 128]

            def z2_tile(t, ch):
                """Same, from the gathered z1 slices."""
                cb, r = divmod(t, MT)
                g, tt = divmod(r, MPG)
                blk = cb * MPG + tt
                return z1_sb[g][:, blk * C + ch * 128: blk * C + ch * 128 + 128]

            def at_tile(t, q):
                """rhs: A^T[n-tile t, m-quarter q] -> [128, 256] bf16."""
                k, tt = divmod(t, TPC)
                return at_sb[q][k][:, tt * MQ:(tt + 1) * MQ]

            h_sb = [hsb_pool.tile([128, M_CORE], dt.bfloat16,
                                  tag=f"h{ch}", name=f"h_sb{ch}")
                    for ch in range(2)]

            def tail(li, qs, h_ps, on_tile_done, prio):
                # weight apply + relu + store for the m-tiles of the
                # just-finished pass; runs overlapped with the next
                # pass's accumulation matmuls.
                with prio:
                    for ch in range(2):
                        for q in qs:
                            nc.vector.tensor_copy(
                                h_sb[ch][:, q * MQ:(q + 1) * MQ],
                                h_ps[ch][q % 2][:],
                            )
                    for i in range(qs[0] * MPG, (qs[-1] + 1) * MPG):
                        z_ps = psz_pool.tile([128, C], dt.float32,
                                             tag="zps",
                                             name=f"z_ps_{li}_{i}")
                        for ch in range(2):
                            nc.tensor.matmul(
                                z_ps[:, ch * 128:(ch + 1) * 128],
                                h_sb[ch][:, i * 128:(i + 1) * 128],
                                w_sb[:, li * 128:(li + 1) * 128],
                                start=True, stop=True,
                            )
                        z_o = zout_pool.tile([128, C], dt.bfloat16,
                                             tag="zo", name=f"z_o_{li}_{i}")
                        nc.scalar.activation(z_o[:], z_ps[:],
                                             mybir.ActivationFunctionType.Relu)
                        on_tile_done(i, z_o)

            # Warm the ncfw collective path with a full-size RDH gather
            # (same shape as the real slices), hidden under the A load.
            nc.gpsimd.dma_start(out=warm_in[:], in_=z0_d[0:MPG * 128, :])
            nc.gpsimd.collective_compute(
                "AllGather",
                mybir.AluOpType.bypass,
                replica_groups=[list(range(NCORES))],
                ins=[warm_in.ap().opt()],
                outs=[warm_out.ap().opt()],
            )

            # ---- layer 1: four m-quarter passes ----
            def l1_tile_done(i, z_o):
                nc.scalar.dma_start(out=z1_loc[i * 128:(i + 1) * 128, :],
                                    in_=z_o[:])
                if i % MPG == MPG - 1:
                    g = i // MPG
                    nc.gpsimd.collective_compute(
                        "AllGather",
                        mybir.AluOpType.bypass,
                        replica_groups=[list(range(NCORES))],
                        ins=[z1_loc.ap()[g * MPG * 128:(g + 1) * MPG * 128, :].opt()],
                        outs=[z1g[g].ap().opt()],
                    )
                    nc.sync.dma_start(
                        out=z1_sb[g].rearrange("p (cb t c) -> p cb t c",
                                               cb=NCORES, t=MPG),
                        in_=z1g[g].ap().rearrange("(cb t p) c -> p cb t c",
                                                  cb=NCORES, p=128))

            for q in range(NQ):
                h_ps = {}
                for ch in range(2):
                    h_ps[ch] = {q % 2: psh_pool.tile(
                        [128, MQ], dt.float32, tag=f"hps{ch}{q % 2}",
                        name=f"hps_0_{ch}_{q}")}
                for ti, t in enumerate(range(NT)):
                    for ch in range(2):
                        nc.tensor.matmul(
                            h_ps[ch][q % 2][:],
                            z_tile(t, ch),
                            at_tile(t, q),
                            start=(ti == 0),
                            stop=(ti == NT - 1),
                        )
                tail(0, [q], h_ps, l1_tile_done, tc.high_priority())

            # ---- layer 2: two m-half passes, n-tiles in gather order ----
            t2 = [MT * cb + MPG * g + tt
                  for g in range(NQ) for cb in range(NCORES)
                  for tt in range(MPG)]

            def l2_tile_done(i, z_o):
                nc.sync.dma_start(out=out_d[i * 128:(i + 1) * 128, :],
                                  in_=z_o[:])

            for mh in range(2):
                qs = (2 * mh, 2 * mh + 1)
                h_ps = {ch: {q % 2: psh_pool.tile(
                    [128, MQ], dt.float32, tag=f"hps{ch}{q % 2}",
                    name=f"hps_1_{ch}_{q}") for q in qs}
                    for ch in range(2)}
                with tc.high_priority(offset=-1_000_000):
                    for ti, t in enumerate(t2):
                        for ch in range(2):
                            for q in qs:
                                nc.tensor.matmul(
                                    h_ps[ch][q % 2][:],
                                    z2_tile(t, ch),
                                    at_tile(t, q),
                                    start=(ti == 0),
                                    stop=(ti == NT - 1),
                                )
                tail(1, list(qs), h_ps, l2_tile_done,
                     contextlib.nullcontext())

    nc.compile()
    return nc


def _prep_inputs(x, net_params, A):
    a_bf = A.astype(BF16)
    z0 = np.ascontiguousarray(x.transpose(1, 0, 2).reshape(N, C)).astype(BF16)
    w = net_params.astype(np.float32).reshape(L, F, F).astype(BF16)
    # block-diagonal weight tile per layer: diag(W_l, W_l)
    w_sb = np.zeros((128, 2 * 128), dtype=BF16)
    for li in range(L):
        w_sb[0:F, li * 128:li * 128 + F] = w[li]
        w_sb[F:2 * F, li * 128 + F:li * 128 + 2 * F] = w[li]
    in_maps = []
    for j in range(NCORES):
        at_j = np.ascontiguousarray(a_bf[j * M_CORE:(j + 1) * M_CORE, :].T)
        # m-quarter-major so each (quarter, chunk) DMA is contiguous
        at_q = np.ascontiguousarray(
            np.stack([at_j[:, q * MQ:(q + 1) * MQ] for q in range(NQ)])
        ).reshape(NQ * N, MQ)
        in_maps.append({"at": at_q, "z0": z0, "w": w_sb})
    return in_maps


def kernel(x, t, net_params, A):
    x = np.asarray(x)
    A = np.asarray(A)
    net_params = np.asarray(net_params)

    if "nc" not in _CACHED:
        _CACHED["nc"] = _build_program()
    nc = _CACHED["nc"]

    in_maps = _prep_inputs(x, net_params, A)
    _CACHED["in_maps"] = in_maps
    res = run_bass_kernel_spmd(nc, in_maps, list(range(NCORES)))
    full = np.concatenate([res.results[c]["out"] for c in range(NCORES)],
                          axis=0).astype(np.float32)
    return np.ascontiguousarray(full.reshape(N, B, F).transpose(1, 0, 2))


# revision 9
# speedup vs baseline: 1.1013x; 1.1013x over previous
"""Trainium2 Bass kernel for a 2-layer dense GCN block:

    z = x.reshape(B, N, F)                     # B=4, N=8192, F=64
    for i in range(2):
        z = relu((A @ z) @ W_i)                # A: [N, N] dense
    return z

Strategy (8 NeuronCores, SPMD):
  * Shard the output rows (m) of A @ Z across cores: core j owns rows
    [1024*j, 1024*(j+1)).  The host hands core j the matching
    column-slice of A^T (contraction dim n on SBUF partitions), cast to
    bf16 and pre-swizzled m-quarter-major so every DMA chunk is fully
    contiguous in DRAM.  The 16 MB shard stays resident in SBUF for
    BOTH layers -- A is read from HBM exactly once, at full ring rate
    on the sync HWDGE ring (z0 / z1 traffic uses the scalar ring).
  * Z is a [n, c] matrix with c = b*F + f (256 columns).  Layer matmuls
    compute H^T[c, m] = sum_n Z[n, c] * A^T[n, m] on the tensor engine
    (lhsT = Z tile stationary, rhs = A^T tile moving, fp32 PSUM accum).
    The n-loop is outermost, in DMA-chunk order.
  * Layer 1 runs as FOUR m-quarter passes (256 m columns each).  A
    quarter's rows finish mid-layer, so its AllGather (the only
    inter-layer exchange) triggers as early as ~1/4 through layer 1 and
    the whole 4-slice gather chain pipelines under the rest of layer 1
    and the start of layer 2 -- the collective's ~20us ncfw latency is
    completely hidden and the PE never idles between layers (idle >
    ~3.4us would also drop the HAM clock gate to half rate).
  * Gather slices land in their own SBUF tiles (z0 stays live), and the
    reload DMAs ride the sync ring so a reload waiting on its collective
    can never block a later z1_loc store (scalar ring) whose gather
    trigger is on the critical path.
  * Layer 2 accumulation matmuls are emitted at LOW scheduler priority
    so layer-1 tail work (weight-apply -> relu -> store -> gather
    trigger) always precedes them in the PE queue -- otherwise a
    reload-waiting LDWEIGHTS can wedge the whole PE FIFO.
  * Layer 2 consumes gather slices in arrival order, in two m-half
    passes whose output stores overlap the remaining matmuls.
  * bf16 operands / fp32 accumulation throughout (~0.5% rel-l2 vs the
    fp32 reference).
"""

import contextlib

import numpy as np
import ml_dtypes

import concourse.mybir as mybir
import concourse.tile as tile
from concourse import bacc
from concourse.bass_utils import run_bass_kernel_spmd

BF16 = ml_dtypes.bfloat16

NCORES = 8
B, N, F, L = 4, 8192, 64, 2
C = B * F                      # 256 columns of the Z matrix
M_CORE = N // NCORES           # 1024 output rows per core
NT = N // 128                  # 64 contraction tiles of 128
MT = M_CORE // 128             # 8 output-row tiles of 128 per core
KCH = 8                        # DMA chunks for the resident A^T shard
TPC = NT // KCH                # 8 n-tiles per chunk
NQ = 4                         # m-quarter passes / gather slices
MPG = MT // NQ                 # m-tiles per gather slice (2)
MQ = M_CORE // NQ              # m columns per quarter (256)

_CACHED = {}


def _build_program():
    nc = bacc.Bacc("TRN2", target_bir_lowering=False, debug=False,
                   num_devices=NCORES)
    dt = mybir.dt

    # Host pre-swizzles A^T and z0 into exact SBUF tile order, so every
    # chunk DMA is one flat contiguous [128, free] copy (4 KB+ runs per
    # partition -- 512 B runs measurably halve HBM throughput).
    at_d = nc.dram_tensor("at", [NQ * KCH * 128, TPC * MQ], dt.bfloat16,
                          kind="ExternalInput")
    z0_d = nc.dram_tensor("z0", [KCH * 128, TPC * C], dt.bfloat16,
                          kind="ExternalInput")
    w_d = nc.dram_tensor("w", [128, 2 * 128], dt.bfloat16, kind="ExternalInput")
    out_d = nc.dram_tensor("out", [M_CORE, C], dt.bfloat16, kind="ExternalOutput")

    z1_loc = nc.dram_tensor("z1_loc", [M_CORE, C], dt.bfloat16)
    warm_in = nc.dram_tensor("warm_in", [MPG * 128, C], dt.bfloat16)
    warm_out = nc.dram_tensor("warm_out", [NCORES * MPG * 128, C], dt.bfloat16)
    z1g = [nc.dram_tensor(f"z1g{g}", [NCORES * MPG * 128, C], dt.bfloat16)
           for g in range(NQ)]

    with tile.TileContext(nc) as tc:
        with tc.tile_pool(name="a_res", bufs=1) as a_pool, \
             tc.tile_pool(name="z_res", bufs=1) as z_pool, \
             tc.tile_pool(name="z1_res", bufs=1) as z1_pool, \
             tc.tile_pool(name="wk", bufs=1) as w_pool, \
             tc.tile_pool(name="ht", bufs=1, space="PSUM") as psh_pool, \
             tc.tile_pool(name="pz", bufs=4, space="PSUM") as psz_pool, \
             tc.tile_pool(name="hsb", bufs=2) as hsb_pool, \
             tc.tile_pool(name="zout", bufs=8) as zout_pool:

            w_sb = w_pool.tile([128, 2 * 128], dt.bfloat16, tag="w")
            nc.scalar.dma_start(out=w_sb[:], in_=w_d[:])

            # Resident A^T shard: one SBUF tile per (quarter, chunk).
            at_sb = [[a_pool.tile([128, TPC * MQ], dt.bfloat16,
                                  tag=f"at{q}_{k}", name=f"at_sb{q}_{k}")
                      for k in range(KCH)] for q in range(NQ)]
            z_sb = [z_pool.tile([128, TPC * C], dt.bfloat16,
                                tag=f"z{k}", name=f"z_sb{k}")
                    for k in range(KCH)]
            z1_sb = [z1_pool.tile([128, NCORES * MPG * C], dt.bfloat16,
                                  tag=f"z1g{g}", name=f"z1_sb{g}")
                     for g in range(NQ)]

            for k in range(KCH):
                nc.scalar.dma_start(out=z_sb[k][:],
                                    in_=z0_d[k * 128:(k + 1) * 128, :])
            for q in range(NQ):
                for k in range(KCH):
                    r = (q * KCH + k) * 128
                    nc.sync.dma_start(out=at_sb[q][k][:],
                                      in_=at_d[r:r + 128, :])

            def z_tile(t, ch):
                """lhsT: Z[n-tile t, c-half ch] -> [128, 128] bf16."""
                k, tt = divmod(t, TPC)
                return z_sb[k][:, tt * C + ch * 128: tt * C + ch * 128 + 128]

            def z2_tile(t, ch):
                """Same, from the gathered z1 slices."""
                cb, r = divmod(t, MT)
                g, tt = divmod(r, MPG)
                blk = cb * MPG + tt
                return z1_sb[g][:, blk * C + ch * 128: blk * C + ch * 128 + 128]

            def at_tile(t, q):
                """rhs: A^T[n-tile t, m-quarter q] -> [128, 256] bf16."""
                k, tt = divmod(t, TPC)
                return at_sb[q][k][:, tt * MQ:(tt + 1) * MQ]

            h_sb = [hsb_pool.tile([128, M_CORE], dt.bfloat16,
                                  tag=f"h{ch}", name=f"h_sb{ch}")
                    for ch in range(2)]

            def tail(li, qs, h_ps, on_tile_done, prio):
                # weight apply + relu + store for the m-tiles of the
                # just-finished pass; runs overlapped with the next
                # pass's accumulation matmuls.
                with prio:
                    for ch in range(2):
                        for q in qs:
                            nc.vector.tensor_copy(
                                h_sb[ch][:, q * MQ:(q + 1) * MQ],
                                h_ps[ch][q % 2][:],
                            )
                    for i in range(qs[0] * MPG, (qs[-1] + 1) * MPG):
                        z_ps = psz_pool.tile([128, C], dt.float32,
                                             tag="zps",
                                             name=f"z_ps_{li}_{i}")
                        for ch in range(2):
                            nc.tensor.matmul(
                                z_ps[:, ch * 128:(ch + 1) * 128],
                                h_sb[ch][:, i * 128:(i + 1) * 128],
                                w_sb[:, li * 128:(li + 1) * 128],
                                start=True, stop=True,
                            )
                        z_o = zout_pool.tile([128, C], dt.bfloat16,
                                             tag="zo", name=f"z_o_{li}_{i}")
                        nc.scalar.activation(z_o[:], z_ps[:],
                                             mybir.ActivationFunctionType.Relu)
                        on_tile_done(i, z_o)

            # Warm the ncfw collective path with a full-size RDH gather
            # (same shape as the real slices), hidden under the A load.
            nc.gpsimd.dma_start(out=warm_in[:],
                                in_=z0_d[0:MPG * 128, 0:C])
            nc.gpsimd.collective_compute(
                "AllGather",
                mybir.AluOpType.bypass,
                replica_groups=[list(range(NCORES))],
                ins=[warm_in.ap().opt()],
                outs=[warm_out.ap().opt()],
            )

            # ---- layer 1: four m-quarter passes ----
            def l1_tile_done(i, z_o):
                nc.scalar.dma_start(out=z1_loc[i * 128:(i + 1) * 128, :],
                                    in_=z_o[:])
                if i % MPG == MPG - 1:
                    g = i // MPG
                    nc.gpsimd.collective_compute(
                        "AllGather",
                        mybir.AluOpType.bypass,
                        replica_groups=[list(range(NCORES))],
                        ins=[z1_loc.ap()[g * MPG * 128:(g + 1) * MPG * 128, :].opt()],
                        outs=[z1g[g].ap().opt()],
                    )
                    nc.sync.dma_start(
                        out=z1_sb[g].rearrange("p (cb t c) -> p cb t c",
                                               cb=NCORES, t=MPG),
                        in_=z1g[g].ap().rearrange("(cb t p) c -> p cb t c",
                                                  cb=NCORES, p=128))

            for q in range(NQ):
                h_ps = {}
                for ch in range(2):
                    h_ps[ch] = {q % 2: psh_pool.tile(
                        [128, MQ], dt.float32, tag=f"hps{ch}{q % 2}",
                        name=f"hps_0_{ch}_{q}")}
                for ti, t in enumerate(range(NT)):
                    for ch in range(2):
                        nc.tensor.matmul(
                            h_ps[ch][q % 2][:],
                            z_tile(t, ch),
                            at_tile(t, q),
                            start=(ti == 0),
                            stop=(ti == NT - 1),
                        )
                tail(0, [q], h_ps, l1_tile_done, tc.high_priority())

            # ---- layer 2: two m-half passes, n-tiles in gather order ----
            t2 = [MT * cb + MPG * g + tt
                  for g in range(NQ) for cb in range(NCORES)
                  for tt in range(MPG)]

            def l2_tile_done(i, z_o):
                nc.sync.dma_start(out=out_d[i * 128:(i + 1) * 128, :],
                                  in_=z_o[:])

            for mh in range(2):
                qs = (2 * mh, 2 * mh + 1)
                h_ps = {ch: {q % 2: psh_pool.tile(
                    [128, MQ], dt.float32, tag=f"hps{ch}{q % 2}",
                    name=f"hps_1_{ch}_{q}") for q in qs}
                    for ch in range(2)}
                with tc.high_priority(offset=-1_000_000):
                    for ti, t in enumerate(t2):
                        for ch in range(2):
                            for q in qs:
                                nc.tensor.matmul(
                                    h_ps[ch][q % 2][:],
                                    z2_tile(t, ch),
                                    at_tile(t, q),
                                    start=(ti == 0),
                                    stop=(ti == NT - 1),
                                )
                tail(1, list(qs), h_ps, l2_tile_done,
                     contextlib.nullcontext())

    nc.compile()
    return nc


def _prep_inputs(x, net_params, A):
    a_bf = A.astype(BF16)
    z0 = np.ascontiguousarray(x.transpose(1, 0, 2).reshape(N, C)).astype(BF16)
    # z0 in SBUF tile order [k, p, t, c] -> [KCH*128, TPC*C]
    z0_sw = np.ascontiguousarray(
        z0.reshape(KCH, TPC, 128, C).transpose(0, 2, 1, 3)
    ).reshape(KCH * 128, TPC * C)
    w = net_params.astype(np.float32).reshape(L, F, F).astype(BF16)
    # block-diagonal weight tile per layer: diag(W_l, W_l)
    w_sb = np.zeros((128, 2 * 128), dtype=BF16)
    for li in range(L):
        w_sb[0:F, li * 128:li * 128 + F] = w[li]
        w_sb[F:2 * F, li * 128 + F:li * 128 + 2 * F] = w[li]
    in_maps = []
    for j in range(NCORES):
        at_j = np.ascontiguousarray(a_bf[j * M_CORE:(j + 1) * M_CORE, :].T)
        # A^T in SBUF tile order [q, k, p, t, m] -> [NQ*KCH*128, TPC*MQ]
        at_sw = np.ascontiguousarray(
            at_j.reshape(KCH, TPC, 128, NQ, MQ).transpose(3, 0, 2, 1, 4)
        ).reshape(NQ * KCH * 128, TPC * MQ)
        in_maps.append({"at": at_sw, "z0": z0_sw, "w": w_sb})
    return in_maps


def kernel(x, t, net_params, A):
    x = np.asarray(x)
    A = np.asarray(A)
    net_params = np.asarray(net_params)

    if "nc" not in _CACHED:
        _CACHED["nc"] = _build_program()
    nc = _CACHED["nc"]

    in_maps = _prep_inputs(x, net_params, A)
    _CACHED["in_maps"] = in_maps
    res = run_bass_kernel_spmd(nc, in_maps, list(range(NCORES)))
    full = np.concatenate([res.results[c]["out"] for c in range(NCORES)],
                          axis=0).astype(np.float32)
    return np.ascontiguousarray(full.reshape(N, B, F).transpose(1, 0, 2))
